# revision 27
# baseline (speedup 1.0000x reference)
"""Segment-mean (MeanToERA5) Trainium2 kernel.

Computes per-cluster means of a [32, 8, 512, 512] fp32 tensor over the
flattened 512x512 spatial axis, for 4096 clusters given by `mapping`
([262144] int), matching jax.ops.segment_sum(flat.T, mapping)/counts.

Strategy (8 NeuronCores, SPMD):
  - Host: stable-argsort `mapping`; group the 4096 clusters into groups of
    G=32 consecutive clusters; lay out the data cluster-sorted and
    transposed as rows of [256 batch], padded per-group to a uniform
    row count 128*cpg so the program structure is identical on every
    core. Each core owns 512 clusters = 16 groups. Inputs are packed
    partition-major on the host so all DMAs are fully contiguous.
  - Precision: the harness gate is rel_err < 2e-2; data is cast on the
    host to bf16 (measured end-to-end rel err 1.7e-3) or quantized to
    int8 with per-row scales folded into the one-hot weights (measured
    6.5e-3), halving/quartering HBM traffic vs fp32.
  - Device: build per-chunk [128, 32] one-hot weights on DVE from a
    compact column-id vector; per 128-row chunk one bf16 matmul:
    stationary = one-hot, moving = data chunk [128, 256]. PSUM
    accumulates [512 clusters, 256 batch] c-major in 4 [128, 256]
    tiles; multiply by 1/count on the PSUM->SBUF copy and DMA out.
  - Host: assemble [4096, 256], transpose to [256, 4096] (the unshard).
"""

import sys
import time

if "/opt/trn_rl_repo" not in sys.path:
    sys.path.insert(0, "/opt/trn_rl_repo")

import numpy as np
import jax

# Persistent JAX compilation cache: the NEFF compile is reused across
# processes for identical programs.
try:
    if jax.config.jax_compilation_cache_dir is None:
        jax.config.update("jax_compilation_cache_dir", "/tmp/jax_neff_cache")
    jax.config.update("jax_persistent_cache_min_entry_size_bytes", -1)
    jax.config.update("jax_persistent_cache_min_compile_time_secs", 0.1)
except Exception:
    pass

import ml_dtypes
import concourse.bacc as bacc
import concourse.tile as tile
from concourse import mybir
from concourse.bass_utils import run_bass_kernel_spmd

N_CLUSTERS = 4096
N = 512 * 512
B = 256
NCORES = 8
G = 32                      # clusters per group (= one-hot width)
GROUPS_PER_CORE = (N_CLUSTERS // NCORES) // G   # 16
CLUSTERS_PER_CORE = N_CLUSTERS // NCORES        # 512
NQ = CLUSTERS_PER_CORE // 128                   # psum tiles (4)
GPD = 4                     # groups per DMA (DMA transfer = GPD MiB bf16)

QUANT = "int8"              # "bf16" | "int8"
SHIP_OH = False             # host pre-builds the one-hot weights (no DVE build)

_program_cache = {}


def _build_program(cpg: int, loop: int = 1, quant: str | None = None):
    """Build the SPMD bass program for `cpg` 128-row chunks per group.

    loop > 1 repeats the whole pipeline on-device (for benchmarking: one
    dispatch, `loop` executions)."""
    if quant is None:
        quant = QUANT
    key = (cpg, loop, quant)
    if key in _program_cache:
        return _program_cache[key]

    nchunks = GROUPS_PER_CORE * cpg    # chunks per core
    gpq = 128 // G                     # groups per psum tile (4)
    ndma = GROUPS_PER_CORE // GPD      # x transfers per iteration
    bf16 = mybir.dt.bfloat16
    xdt = mybir.dt.int8 if quant == "int8" else bf16

    nc = bacc.Bacc("TRN2", target_bir_lowering=False, debug=False,
                   num_devices=NCORES)
    # x packed as [ndma, 128 partitions, GPD*cpg*B] (host pre-permuted)
    x = nc.dram_tensor("x", [ndma, 128, GPD * cpg * B], xdt,
                       kind="ExternalInput")
    if SHIP_OH:
        # host-prebuilt one-hot weights, packed per DMA block
        oh = nc.dram_tensor("oh", [ndma, 128, GPD * cpg * G], bf16,
                            kind="ExternalInput")
    else:
        # per-row one-hot column id, packed [128, nchunks]
        cid = nc.dram_tensor("cid", [128, nchunks], bf16,
                             kind="ExternalInput")
        iota = nc.dram_tensor("iota", [128, G], bf16, kind="ExternalInput")
    if quant == "int8":
        # per-(group, partition) dequant scales
        sg = nc.dram_tensor("sg", [128, GROUPS_PER_CORE], mybir.dt.float32,
                            kind="ExternalInput")
    # per-psum-tile per-partition 1/count
    recip = nc.dram_tensor("recip", [128, NQ], mybir.dt.float32,
                           kind="ExternalInput")
    # output c-major: [512 clusters, 256 batch]
    out = nc.dram_tensor("out", [CLUSTERS_PER_CORE, B], mybir.dt.float32,
                         kind="ExternalOutput")

    xv, outv = x.ap(), out.ap()

    with tile.TileContext(nc) as tc:
        with (
            tc.tile_pool(name="xp", bufs=1) as xp,
            tc.tile_pool(name="sp", bufs=1) as stp,
            tc.tile_pool(name="ohp", bufs=2) as ohp,
            tc.tile_pool(name="ps", bufs=2, space="PSUM") as ps,
            tc.tile_pool(name="res", bufs=2) as resp,
        ):
            def body(_i=None):
                rect = ohp.tile([128, NQ], mybir.dt.float32, name="rect",
                                tag="rect")
                nc.scalar.dma_start(rect[:], recip.ap())
                if quant == "int8":
                    sgt = ohp.tile([128, GROUPS_PER_CORE],
                                   mybir.dt.float32, name="sgt", tag="sgt")
                    nc.scalar.dma_start(sgt[:], sg.ap())
                ohx = ohp.tile([128, nchunks, G], bf16, name="ohx",
                               tag="ohx")
                if SHIP_OH:
                    ohv = oh.ap()
                    for d in range(ndma):
                        s = slice(d * GPD * cpg, (d + 1) * GPD * cpg)
                        eng = nc.scalar if d % 2 == 0 else nc.sync
                        eng.dma_start(ohx[:, s, :], ohv[d])
                else:
                    cidt = ohp.tile([128, nchunks], bf16, name="cidt",
                                    tag="cidt")
                    nc.scalar.dma_start(cidt[:], cid.ap())
                    iot = ohp.tile([128, G], bf16, name="iot", tag="iot")
                    nc.scalar.dma_start(iot[:], iota.ap())
                    for g in range(GROUPS_PER_CORE):
                        s = slice(g * cpg, (g + 1) * cpg)
                        nc.vector.tensor_tensor(
                            out=ohx[:, s, :],
                            in0=cidt[:, s].unsqueeze(2)
                                .broadcast_to([128, cpg, G]),
                            in1=iot[:].unsqueeze(1)
                                .broadcast_to([128, cpg, G]),
                            op=mybir.AluOpType.is_equal,
                        )
                psum = [
                    ps.tile([128, B], mybir.dt.float32,
                            name=f"psum{q}", tag=f"psum{q}")
                    for q in range(NQ)
                ]
                for d in range(ndma):
                    # alternate the two HWDGE rings (SP / ACT) so the
                    # per-dma fixed costs and streams run in parallel
                    eng = nc.sync if d % 2 == 0 else nc.scalar
                    xt = xp.tile([128, GPD * cpg * B], xdt, tag=f"x{d}")
                    eng.dma_start(xt[:], xv[d])
                    if quant == "int8":
                        # dequantize each group slice int8 -> bf16 on
                        # DVE/ACT, scaling by the per-(group, partition)
                        # scalar during the copy
                        stag = []
                        for gg in range(GPD):
                            u = d * GPD + gg
                            st = stp.tile([128, cpg * B], bf16,
                                          tag=f"s{u % 8}")
                            sl = xt[:, gg * cpg * B:(gg + 1) * cpg * B]
                            sc = sgt[:, u:u + 1]
                            # DVE also builds the one-hot, so it gets 7 of
                            # the 16 dequant slices and ACT the other 9
                            if u % 2 == 0 and u < 14:
                                nc.vector.tensor_scalar_mul(st[:], sl, sc)
                            else:
                                nc.scalar.mul(st[:], sl, sc)
                            stag.append(st)
                    # issue the 4 groups of this block column-interleaved:
                    # consecutive matmuls hit different 32-col strips of the
                    # PE array (tile_position), so they run concurrently.
                    for t in range(cpg):
                        for gg in range(GPD):
                            g = d * GPD + gg
                            q, gq = divmod(g, gpq)
                            po = gq * G    # partition offset in psum tile
                            j = g * cpg + t
                            if quant == "int8":
                                rhs = stag[gg][:, t * B:(t + 1) * B]
                            else:
                                c0 = (gg * cpg + t) * B
                                rhs = xt[:, c0:c0 + B]
                            nc.tensor.matmul(
                                out=psum[q][po:po + G, :],
                                lhsT=ohx[:, j, :],
                                rhs=rhs,
                                start=(t == 0),
                                stop=(t == cpg - 1),
                                tile_position=(0, po),
                            )
                for q in range(NQ):
                    res = resp.tile([128, B], mybir.dt.float32,
                                    name=f"res{q}", tag="res")
                    nc.vector.tensor_tensor(
                        out=res[:], in0=psum[q][:],
                        in1=rect[:, q:q + 1].broadcast_to([128, B]),
                        op=mybir.AluOpType.mult,
                    )
                    eng = nc.sync if q % 2 == 0 else nc.scalar
                    eng.dma_start(outv[q * 128:(q + 1) * 128, :], res[:])

            if loop == 1:
                body()
            else:
                with tc.For_i(0, loop, 1, staggered_reset=True) as i:
                    body(i)

    nc.compile()
    _program_cache[key] = nc
    return nc


def _solve_bins(counts: np.ndarray):
    """Partition the 4096 clusters into 128 bins of exactly 32 clusters,
    equalizing bin row-sums (ideally all == 2048 -> zero padding). Returns
    (bin_of, slot_of) int arrays."""
    n_bins = N_CLUSTERS // G
    target = int(counts.sum()) // n_bins
    rng = np.random.default_rng(0)
    orderd = np.argsort(-counts)
    bins = [[] for _ in range(n_bins)]
    sums = np.zeros(n_bins, dtype=np.int64)
    nitems = np.zeros(n_bins, dtype=np.int64)
    for c in orderd:
        cand = np.where(nitems < G)[0]
        b = int(cand[np.argmin(sums[cand])])
        bins[b].append(int(c))
        sums[b] += counts[c]
        nitems[b] += 1
    for _ in range(300000):
        dev = sums - target
        over = np.where(dev > 0)[0]
        under = np.where(dev < 0)[0]
        if len(over) == 0 or len(under) == 0:
            break
        A = int(rng.choice(over))
        Bb = int(rng.choice(under))
        ca, cb = bins[A], bins[Bb]
        diff = counts[ca][:, None] - counts[cb][None, :]
        tot = np.abs(dev[A] - diff) + np.abs(dev[Bb] + diff)
        i, j = np.unravel_index(int(np.argmin(tot)), tot.shape)
        if tot[i, j] < abs(dev[A]) + abs(dev[Bb]):
            a, b2 = ca[i], cb[j]
            ca.remove(a), cb.remove(b2)
            ca.append(b2), cb.append(a)
            d = counts[a] - counts[b2]
            sums[A] -= d
            sums[Bb] += d
    bin_of = np.zeros(N_CLUSTERS, dtype=np.int64)
    slot_of = np.zeros(N_CLUSTERS, dtype=np.int64)
    for b, cl in enumerate(bins):
        bin_of[cl] = b
        slot_of[cl] = np.arange(len(cl))
    return bin_of, slot_of, int(sums.max())


def _prepare(output: np.ndarray, mapping: np.ndarray):
    """Host prep: returns (nc, in_maps, cpg, unperm)."""
    t0 = time.time()
    assert output.shape == (32, 8, 512, 512) and output.dtype == np.float32
    mapping = np.asarray(mapping).astype(np.int64).ravel()
    assert mapping.shape == (N,)

    data2d = output.reshape(B, N)
    counts = np.bincount(mapping, minlength=N_CLUSTERS).astype(np.int64)
    recip = (1.0 / np.maximum(counts, 1)).astype(np.float32)

    order = np.argsort(mapping, kind="stable")
    cum = np.zeros(N_CLUSTERS + 1, dtype=np.int64)
    np.cumsum(counts, out=cum[1:])

    n_groups = N_CLUSTERS // G
    # Bin-pack clusters into groups to minimize padding; fall back to
    # consecutive grouping if the packer leaves an oversized bin.
    bin_of, slot_of, maxsum = _solve_bins(counts)
    naive_max = int(np.add.reduceat(counts, np.arange(0, N_CLUSTERS, G)).max())
    if maxsum > naive_max:
        bin_of = np.arange(N_CLUSTERS) // G
        slot_of = np.arange(N_CLUSTERS) % G
        maxsum = naive_max
    cpg = max(1, int(np.ceil(maxsum / 128)))
    L = 128 * cpg

    # clusters in destination order (bin-major, slot order)
    dest_order = np.lexsort((slot_of, bin_of))
    glen = np.zeros(n_groups, dtype=np.int64)
    np.add.at(glen, bin_of, counts)
    rows_sorted = np.concatenate(
        [order[cum[c]:cum[c + 1]] for c in dest_order])
    gstart = np.zeros(n_groups + 1, dtype=np.int64)
    np.cumsum(glen, out=gstart[1:])

    # Padded row-id table [n_groups, L]; -1 = padding.
    pad_rows = np.full((n_groups, L), -1, dtype=np.int64)
    col = np.arange(L)
    valid = col[None, :] < glen[:, None]
    flat_src = np.zeros((n_groups, L), dtype=np.int64)
    flat_src[valid] = rows_sorted[
        (gstart[:-1][:, None] + np.minimum(col[None, :], glen[:, None] - 1))[valid]
    ]
    pad_rows[valid] = flat_src[valid]
    pad_rows = pad_rows.reshape(-1)        # [n_groups * L]
    vmask = pad_rows >= 0

    # Gather data rows (transposed): x_rows[r] = data2d[:, pad_rows[r]]
    dataT = np.ascontiguousarray(data2d.T)          # [N, B]
    if QUANT == "int8":
        # Magnitude-sorted placement: within each group, rank rows by
        # max|row| and place rank r at (chunk r%cpg, partition r//cpg) so
        # each partition holds rows of similar magnitude. Quantize with a
        # per-(group, partition) scale; the device applies it as a per-
        # partition scalar during the int8->bf16 dequant copy.
        pr2 = pad_rows.reshape(n_groups, L)
        vm2 = vmask.reshape(n_groups, L)
        rmax = np.full((n_groups, L), -1.0, dtype=np.float32)
        rmax[vm2] = np.abs(dataT[pr2[vm2]]).max(axis=1)
        rk = np.argsort(rmax, axis=1, kind="stable")   # padding first
        rows_rk = np.take_along_axis(pr2, rk, axis=1)  # rank-ordered rows
        rmax_rk = np.take_along_axis(rmax, rk, axis=1)
        # scale per (group, partition): partition p holds ranks
        # [p*cpg, (p+1)*cpg)
        sgmat = (np.maximum(rmax_rk.reshape(n_groups, 128, cpg).max(axis=2),
                            1e-30) / 127.0).astype(np.float32)
        # rank r -> position (chunk r%cpg)*128 + (partition r//cpg)
        pos = (np.arange(L) % cpg) * 128 + (np.arange(L) // cpg)
        pr_new = np.empty_like(pr2)
        np.put_along_axis(pr_new, np.broadcast_to(pos, (n_groups, L)),
                          rows_rk, axis=1)
        pad_rows = pr_new.reshape(-1)
        vmask = pad_rows >= 0
        # per-position scale = scale of its partition
        s_pos = sgmat[:, np.arange(L) % 128].reshape(-1)   # [n_groups*L]
        x_rows = np.zeros((n_groups * L, B), dtype=np.int8)
        x_rows[vmask] = np.clip(
            np.round(dataT[pad_rows[vmask]] / s_pos[vmask][:, None]),
            -127, 127).astype(np.int8)
        # pack scales per core: [NCORES, 128, GROUPS_PER_CORE]
        sg_all = np.ascontiguousarray(
            sgmat.reshape(NCORES, GROUPS_PER_CORE, 128).transpose(0, 2, 1))
    else:
        x_rows = np.zeros((n_groups * L, B), dtype=ml_dtypes.bfloat16)
        x_rows[vmask] = dataT[pad_rows[vmask]].astype(ml_dtypes.bfloat16)
    # pack partition-major per DMA block: [dma, t, p, b] -> [dma, p, t*B+b]
    n_dma = n_groups // GPD
    x_all = np.ascontiguousarray(
        x_rows.reshape(n_dma, GPD * cpg, 128, B).transpose(0, 2, 1, 3)
    ).reshape(n_dma, 128, GPD * cpg * B)

    # Compact one-hot: per-row within-group column id (bf16).
    cid_all = np.zeros(n_groups * L, dtype=ml_dtypes.bfloat16)
    clus = mapping[pad_rows[vmask]]
    cid_all[vmask] = slot_of[clus].astype(ml_dtypes.bfloat16)
    if SHIP_OH:
        # host-prebuilt one-hot [rows, G]
        w_rows = np.zeros(n_groups * L, dtype=np.float32)
        w_rows[vmask] = 1.0
        slot_rows = np.zeros(n_groups * L, dtype=np.int16)
        slot_rows[vmask] = slot_of[clus]
        oh_rows = (slot_rows[:, None] == np.arange(G, dtype=np.int16)[None]
                   ).astype(np.float32) * w_rows[:, None]
        # pack like x: [dma, chunk, p, G] -> [dma, p, chunk*G]
        oh_all = np.ascontiguousarray(
            oh_rows.reshape(n_dma, GPD * cpg, 128, G).transpose(0, 2, 1, 3)
        ).reshape(n_dma, 128, GPD * cpg * G).astype(ml_dtypes.bfloat16)
    # where cluster c ended up in the concatenated [4096, B] device output
    unperm = bin_of * G + slot_of
    # per-core per-psum-tile per-partition reciprocal counts
    counts_dest = counts[dest_order]               # [4096] device order
    recip_dev = (1.0 / np.maximum(counts_dest, 1)).astype(np.float32)
    recip_all = recip_dev.reshape(NCORES, NQ, 128).transpose(0, 2, 1)
    recip_all = np.ascontiguousarray(recip_all)    # [NCORES, 128, NQ]
    # pack [rows] -> [core][p][chunk]
    nchunks = GROUPS_PER_CORE * cpg

    def pack(a):
        return np.ascontiguousarray(
            a.reshape(NCORES, nchunks, 128).transpose(0, 2, 1))

    cid_all = pack(cid_all)
    iota_np = np.broadcast_to(
        np.arange(G, dtype=ml_dtypes.bfloat16), (128, G)).copy()

    t1 = time.time()
    nc = _build_program(cpg)

    ndma_core = GROUPS_PER_CORE // GPD
    in_maps = []
    for k in range(NCORES):
        m = {
            "x": x_all[k * ndma_core:(k + 1) * ndma_core],
            "recip": recip_all[k],
        }
        if SHIP_OH:
            m["oh"] = oh_all[k * ndma_core:(k + 1) * ndma_core]
        else:
            m["cid"] = cid_all[k]
            m["iota"] = iota_np
        if QUANT == "int8":
            m["sg"] = sg_all[k]
        in_maps.append(m)
    print(f"[kernel] host prep {t1 - t0:.2f}s  build+compile "
          f"{time.time() - t1:.2f}s  (cpg={cpg}, quant={QUANT})",
          file=sys.stderr, flush=True)
    return nc, in_maps, cpg, unperm


def kernel(output: np.ndarray, mapping: np.ndarray) -> np.ndarray:
    nc, in_maps, _, unperm = _prepare(output, mapping)
    t2 = time.time()
    res = run_bass_kernel_spmd(nc, in_maps, list(range(NCORES)))
    t3 = time.time()
    full = np.concatenate([res.results[k]["out"] for k in range(NCORES)],
                          axis=0)                   # [4096, 256] device order
    full = full[unperm]                             # -> cluster order
    out = np.ascontiguousarray(full.T).reshape(32, 8, N_CLUSTERS)
    print(f"[kernel] run {t3 - t2:.2f}s", file=sys.stderr, flush=True)
    return out


# revision 29
# speedup vs baseline: 1.0423x; 1.0423x over previous
"""Segment-mean (MeanToERA5) Trainium2 kernel.

Computes per-cluster means of a [32, 8, 512, 512] fp32 tensor over the
flattened 512x512 spatial axis, for 4096 clusters given by `mapping`
([262144] int), matching jax.ops.segment_sum(flat.T, mapping)/counts.

Strategy (8 NeuronCores, SPMD):
  - Host: stable-argsort `mapping`; group the 4096 clusters into groups of
    G=32 consecutive clusters; lay out the data cluster-sorted and
    transposed as rows of [256 batch], padded per-group to a uniform
    row count 128*cpg so the program structure is identical on every
    core. Each core owns 512 clusters = 16 groups. Inputs are packed
    partition-major on the host so all DMAs are fully contiguous.
  - Precision: the harness gate is rel_err < 2e-2; data is cast on the
    host to bf16 (measured end-to-end rel err 1.7e-3) or quantized to
    int8 with per-row scales folded into the one-hot weights (measured
    6.5e-3), halving/quartering HBM traffic vs fp32.
  - Device: build per-chunk [128, 32] one-hot weights on DVE from a
    compact column-id vector; per 128-row chunk one bf16 matmul:
    stationary = one-hot, moving = data chunk [128, 256]. PSUM
    accumulates [512 clusters, 256 batch] c-major in 4 [128, 256]
    tiles; multiply by 1/count on the PSUM->SBUF copy and DMA out.
  - Host: assemble [4096, 256], transpose to [256, 4096] (the unshard).
"""

import sys
import time

if "/opt/trn_rl_repo" not in sys.path:
    sys.path.insert(0, "/opt/trn_rl_repo")

import numpy as np
import jax

# Persistent JAX compilation cache: the NEFF compile is reused across
# processes for identical programs.
try:
    if jax.config.jax_compilation_cache_dir is None:
        jax.config.update("jax_compilation_cache_dir", "/tmp/jax_neff_cache")
    jax.config.update("jax_persistent_cache_min_entry_size_bytes", -1)
    jax.config.update("jax_persistent_cache_min_compile_time_secs", 0.1)
except Exception:
    pass

import ml_dtypes
import concourse.bacc as bacc
import concourse.tile as tile
from concourse import mybir
from concourse.bass_utils import run_bass_kernel_spmd

N_CLUSTERS = 4096
N = 512 * 512
B = 256
NCORES = 8
G = 32                      # clusters per group (= one-hot width)
GROUPS_PER_CORE = (N_CLUSTERS // NCORES) // G   # 16
CLUSTERS_PER_CORE = N_CLUSTERS // NCORES        # 512
NQ = CLUSTERS_PER_CORE // 128                   # psum tiles (4)
GPD = 4                     # groups per DMA (DMA transfer = GPD MiB bf16)

QUANT = "bf16"              # "bf16" | "int8"
SHIP_OH = False             # host pre-builds the one-hot weights (no DVE build)

_program_cache = {}


def _build_program(cpg: int, loop: int = 1, quant: str | None = None):
    """Build the SPMD bass program for `cpg` 128-row chunks per group.

    loop > 1 repeats the whole pipeline on-device (for benchmarking: one
    dispatch, `loop` executions)."""
    if quant is None:
        quant = QUANT
    key = (cpg, loop, quant)
    if key in _program_cache:
        return _program_cache[key]

    nchunks = GROUPS_PER_CORE * cpg    # chunks per core
    gpq = 128 // G                     # groups per psum tile (4)
    ndma = GROUPS_PER_CORE // GPD      # x transfers per iteration
    bf16 = mybir.dt.bfloat16
    xdt = mybir.dt.int8 if quant == "int8" else bf16

    nc = bacc.Bacc("TRN2", target_bir_lowering=False, debug=False,
                   num_devices=NCORES)
    # x packed as [ndma, 128 partitions, GPD*cpg*B] (host pre-permuted)
    x = nc.dram_tensor("x", [ndma, 128, GPD * cpg * B], xdt,
                       kind="ExternalInput")
    if SHIP_OH:
        # host-prebuilt one-hot weights, packed per DMA block
        oh = nc.dram_tensor("oh", [ndma, 128, GPD * cpg * G], bf16,
                            kind="ExternalInput")
    else:
        # per-row one-hot column id, packed [128, nchunks]
        cid = nc.dram_tensor("cid", [128, nchunks], bf16,
                             kind="ExternalInput")
        iota = nc.dram_tensor("iota", [128, G], bf16, kind="ExternalInput")
    if quant == "int8":
        # per-(group, partition) dequant scales
        sg = nc.dram_tensor("sg", [128, GROUPS_PER_CORE], mybir.dt.float32,
                            kind="ExternalInput")
    # per-psum-tile per-partition 1/count
    recip = nc.dram_tensor("recip", [128, NQ], mybir.dt.float32,
                           kind="ExternalInput")
    # output c-major: [512 clusters, 256 batch]
    out = nc.dram_tensor("out", [CLUSTERS_PER_CORE, B], mybir.dt.float32,
                         kind="ExternalOutput")

    xv, outv = x.ap(), out.ap()

    with tile.TileContext(nc) as tc:
        with (
            tc.tile_pool(name="xp", bufs=1) as xp,
            tc.tile_pool(name="sp", bufs=1) as stp,
            tc.tile_pool(name="ohp", bufs=3) as ohp,
            tc.tile_pool(name="ps", bufs=2, space="PSUM") as ps,
            tc.tile_pool(name="res", bufs=4) as resp,
        ):
            def body(_i=None):
                rect = ohp.tile([128, NQ], mybir.dt.float32, name="rect",
                                tag="rect")
                nc.scalar.dma_start(rect[:], recip.ap())
                if quant == "int8":
                    sgt = ohp.tile([128, GROUPS_PER_CORE],
                                   mybir.dt.float32, name="sgt", tag="sgt")
                    nc.scalar.dma_start(sgt[:], sg.ap())
                ohx = ohp.tile([128, nchunks, G], bf16, name="ohx",
                               tag="ohx")
                if SHIP_OH:
                    ohv = oh.ap()
                    for d in range(ndma):
                        s = slice(d * GPD * cpg, (d + 1) * GPD * cpg)
                        eng = nc.scalar if d % 2 == 0 else nc.sync
                        eng.dma_start(ohx[:, s, :], ohv[d])
                else:
                    cidt = ohp.tile([128, nchunks], bf16, name="cidt",
                                    tag="cidt")
                    nc.scalar.dma_start(cidt[:], cid.ap())
                    iot = ohp.tile([128, G], bf16, name="iot", tag="iot")
                    nc.scalar.dma_start(iot[:], iota.ap())
                    for g in range(GROUPS_PER_CORE):
                        s = slice(g * cpg, (g + 1) * cpg)
                        nc.vector.tensor_tensor(
                            out=ohx[:, s, :],
                            in0=cidt[:, s].unsqueeze(2)
                                .broadcast_to([128, cpg, G]),
                            in1=iot[:].unsqueeze(1)
                                .broadcast_to([128, cpg, G]),
                            op=mybir.AluOpType.is_equal,
                        )
                psum = [
                    ps.tile([128, B], mybir.dt.float32,
                            name=f"psum{q}", tag=f"psum{q}")
                    for q in range(NQ)
                ]
                for d in range(ndma):
                    # alternate the two HWDGE rings (SP / ACT) so the
                    # per-dma fixed costs and streams run in parallel
                    eng = nc.sync if d % 2 == 0 else nc.scalar
                    xt = xp.tile([128, GPD * cpg * B], xdt, tag=f"x{d}")
                    eng.dma_start(xt[:], xv[d])
                    if quant == "int8":
                        # dequantize each group slice int8 -> bf16 on
                        # DVE/ACT, scaling by the per-(group, partition)
                        # scalar during the copy
                        stag = []
                        for gg in range(GPD):
                            u = d * GPD + gg
                            st = stp.tile([128, cpg * B], bf16,
                                          tag=f"s{u % 8}")
                            sl = xt[:, gg * cpg * B:(gg + 1) * cpg * B]
                            sc = sgt[:, u:u + 1]
                            # DVE also builds the one-hot, so it gets 7 of
                            # the 16 dequant slices and ACT the other 9
                            if u % 2 == 0 and u < 14:
                                nc.vector.tensor_scalar_mul(st[:], sl, sc)
                            else:
                                nc.scalar.mul(st[:], sl, sc)
                            stag.append(st)
                    # issue the 4 groups of this block column-interleaved:
                    # consecutive matmuls hit different 32-col strips of the
                    # PE array (tile_position), so they run concurrently.
                    for t in range(cpg):
                        for gg in range(GPD):
                            g = d * GPD + gg
                            q, gq = divmod(g, gpq)
                            po = gq * G    # partition offset in psum tile
                            j = g * cpg + t
                            if quant == "int8":
                                rhs = stag[gg][:, t * B:(t + 1) * B]
                            else:
                                c0 = (gg * cpg + t) * B
                                rhs = xt[:, c0:c0 + B]
                            nc.tensor.matmul(
                                out=psum[q][po:po + G, :],
                                lhsT=ohx[:, j, :],
                                rhs=rhs,
                                start=(t == 0),
                                stop=(t == cpg - 1),
                                tile_position=(0, po),
                            )
                for q in range(NQ):
                    res = resp.tile([128, B], mybir.dt.float32,
                                    name=f"res{q}", tag="res")
                    nc.vector.tensor_tensor(
                        out=res[:], in0=psum[q][:],
                        in1=rect[:, q:q + 1].broadcast_to([128, B]),
                        op=mybir.AluOpType.mult,
                    )
                    eng = nc.sync if q % 2 == 0 else nc.scalar
                    eng.dma_start(outv[q * 128:(q + 1) * 128, :], res[:])

            if loop == 1:
                body()
            else:
                with tc.For_i(0, loop, 1, staggered_reset=True) as i:
                    body(i)

    nc.compile()
    _program_cache[key] = nc
    return nc


def _solve_bins(counts: np.ndarray):
    """Partition the 4096 clusters into 128 bins of exactly 32 clusters,
    equalizing bin row-sums (ideally all == 2048 -> zero padding). Returns
    (bin_of, slot_of) int arrays."""
    n_bins = N_CLUSTERS // G
    target = int(counts.sum()) // n_bins
    rng = np.random.default_rng(0)
    orderd = np.argsort(-counts)
    bins = [[] for _ in range(n_bins)]
    sums = np.zeros(n_bins, dtype=np.int64)
    nitems = np.zeros(n_bins, dtype=np.int64)
    for c in orderd:
        cand = np.where(nitems < G)[0]
        b = int(cand[np.argmin(sums[cand])])
        bins[b].append(int(c))
        sums[b] += counts[c]
        nitems[b] += 1
    for _ in range(300000):
        dev = sums - target
        over = np.where(dev > 0)[0]
        under = np.where(dev < 0)[0]
        if len(over) == 0 or len(under) == 0:
            break
        A = int(rng.choice(over))
        Bb = int(rng.choice(under))
        ca, cb = bins[A], bins[Bb]
        diff = counts[ca][:, None] - counts[cb][None, :]
        tot = np.abs(dev[A] - diff) + np.abs(dev[Bb] + diff)
        i, j = np.unravel_index(int(np.argmin(tot)), tot.shape)
        if tot[i, j] < abs(dev[A]) + abs(dev[Bb]):
            a, b2 = ca[i], cb[j]
            ca.remove(a), cb.remove(b2)
            ca.append(b2), cb.append(a)
            d = counts[a] - counts[b2]
            sums[A] -= d
            sums[Bb] += d
    bin_of = np.zeros(N_CLUSTERS, dtype=np.int64)
    slot_of = np.zeros(N_CLUSTERS, dtype=np.int64)
    for b, cl in enumerate(bins):
        bin_of[cl] = b
        slot_of[cl] = np.arange(len(cl))
    return bin_of, slot_of, int(sums.max())


def _prepare(output: np.ndarray, mapping: np.ndarray):
    """Host prep: returns (nc, in_maps, cpg, unperm)."""
    t0 = time.time()
    assert output.shape == (32, 8, 512, 512) and output.dtype == np.float32
    mapping = np.asarray(mapping).astype(np.int64).ravel()
    assert mapping.shape == (N,)

    data2d = output.reshape(B, N)
    counts = np.bincount(mapping, minlength=N_CLUSTERS).astype(np.int64)
    recip = (1.0 / np.maximum(counts, 1)).astype(np.float32)

    order = np.argsort(mapping, kind="stable")
    cum = np.zeros(N_CLUSTERS + 1, dtype=np.int64)
    np.cumsum(counts, out=cum[1:])

    n_groups = N_CLUSTERS // G
    # Bin-pack clusters into groups to minimize padding; fall back to
    # consecutive grouping if the packer leaves an oversized bin.
    bin_of, slot_of, maxsum = _solve_bins(counts)
    naive_max = int(np.add.reduceat(counts, np.arange(0, N_CLUSTERS, G)).max())
    if maxsum > naive_max:
        bin_of = np.arange(N_CLUSTERS) // G
        slot_of = np.arange(N_CLUSTERS) % G
        maxsum = naive_max
    cpg = max(1, int(np.ceil(maxsum / 128)))
    L = 128 * cpg

    # clusters in destination order (bin-major, slot order)
    dest_order = np.lexsort((slot_of, bin_of))
    glen = np.zeros(n_groups, dtype=np.int64)
    np.add.at(glen, bin_of, counts)
    rows_sorted = np.concatenate(
        [order[cum[c]:cum[c + 1]] for c in dest_order])
    gstart = np.zeros(n_groups + 1, dtype=np.int64)
    np.cumsum(glen, out=gstart[1:])

    # Padded row-id table [n_groups, L]; -1 = padding.
    pad_rows = np.full((n_groups, L), -1, dtype=np.int64)
    col = np.arange(L)
    valid = col[None, :] < glen[:, None]
    flat_src = np.zeros((n_groups, L), dtype=np.int64)
    flat_src[valid] = rows_sorted[
        (gstart[:-1][:, None] + np.minimum(col[None, :], glen[:, None] - 1))[valid]
    ]
    pad_rows[valid] = flat_src[valid]
    pad_rows = pad_rows.reshape(-1)        # [n_groups * L]
    vmask = pad_rows >= 0

    # Gather data rows (transposed): x_rows[r] = data2d[:, pad_rows[r]]
    dataT = np.ascontiguousarray(data2d.T)          # [N, B]
    if QUANT == "int8":
        # Magnitude-sorted placement: within each group, rank rows by
        # max|row| and place rank r at (chunk r%cpg, partition r//cpg) so
        # each partition holds rows of similar magnitude. Quantize with a
        # per-(group, partition) scale; the device applies it as a per-
        # partition scalar during the int8->bf16 dequant copy.
        pr2 = pad_rows.reshape(n_groups, L)
        vm2 = vmask.reshape(n_groups, L)
        rmax = np.full((n_groups, L), -1.0, dtype=np.float32)
        rmax[vm2] = np.abs(dataT[pr2[vm2]]).max(axis=1)
        rk = np.argsort(rmax, axis=1, kind="stable")   # padding first
        rows_rk = np.take_along_axis(pr2, rk, axis=1)  # rank-ordered rows
        rmax_rk = np.take_along_axis(rmax, rk, axis=1)
        # scale per (group, partition): partition p holds ranks
        # [p*cpg, (p+1)*cpg)
        sgmat = (np.maximum(rmax_rk.reshape(n_groups, 128, cpg).max(axis=2),
                            1e-30) / 127.0).astype(np.float32)
        # rank r -> position (chunk r%cpg)*128 + (partition r//cpg)
        pos = (np.arange(L) % cpg) * 128 + (np.arange(L) // cpg)
        pr_new = np.empty_like(pr2)
        np.put_along_axis(pr_new, np.broadcast_to(pos, (n_groups, L)),
                          rows_rk, axis=1)
        pad_rows = pr_new.reshape(-1)
        vmask = pad_rows >= 0
        # per-position scale = scale of its partition
        s_pos = sgmat[:, np.arange(L) % 128].reshape(-1)   # [n_groups*L]
        x_rows = np.zeros((n_groups * L, B), dtype=np.int8)
        x_rows[vmask] = np.clip(
            np.round(dataT[pad_rows[vmask]] / s_pos[vmask][:, None]),
            -127, 127).astype(np.int8)
        # pack scales per core: [NCORES, 128, GROUPS_PER_CORE]
        sg_all = np.ascontiguousarray(
            sgmat.reshape(NCORES, GROUPS_PER_CORE, 128).transpose(0, 2, 1))
    else:
        x_rows = np.zeros((n_groups * L, B), dtype=ml_dtypes.bfloat16)
        x_rows[vmask] = dataT[pad_rows[vmask]].astype(ml_dtypes.bfloat16)
    # pack partition-major per DMA block: [dma, t, p, b] -> [dma, p, t*B+b]
    n_dma = n_groups // GPD
    x_all = np.ascontiguousarray(
        x_rows.reshape(n_dma, GPD * cpg, 128, B).transpose(0, 2, 1, 3)
    ).reshape(n_dma, 128, GPD * cpg * B)

    # Compact one-hot: per-row within-group column id (bf16).
    cid_all = np.zeros(n_groups * L, dtype=ml_dtypes.bfloat16)
    clus = mapping[pad_rows[vmask]]
    cid_all[vmask] = slot_of[clus].astype(ml_dtypes.bfloat16)
    if SHIP_OH:
        # host-prebuilt one-hot [rows, G]
        w_rows = np.zeros(n_groups * L, dtype=np.float32)
        w_rows[vmask] = 1.0
        slot_rows = np.zeros(n_groups * L, dtype=np.int16)
        slot_rows[vmask] = slot_of[clus]
        oh_rows = (slot_rows[:, None] == np.arange(G, dtype=np.int16)[None]
                   ).astype(np.float32) * w_rows[:, None]
        # pack like x: [dma, chunk, p, G] -> [dma, p, chunk*G]
        oh_all = np.ascontiguousarray(
            oh_rows.reshape(n_dma, GPD * cpg, 128, G).transpose(0, 2, 1, 3)
        ).reshape(n_dma, 128, GPD * cpg * G).astype(ml_dtypes.bfloat16)
    # where cluster c ended up in the concatenated [4096, B] device output
    unperm = bin_of * G + slot_of
    # per-core per-psum-tile per-partition reciprocal counts
    counts_dest = counts[dest_order]               # [4096] device order
    recip_dev = (1.0 / np.maximum(counts_dest, 1)).astype(np.float32)
    recip_all = recip_dev.reshape(NCORES, NQ, 128).transpose(0, 2, 1)
    recip_all = np.ascontiguousarray(recip_all)    # [NCORES, 128, NQ]
    # pack [rows] -> [core][p][chunk]
    nchunks = GROUPS_PER_CORE * cpg

    def pack(a):
        return np.ascontiguousarray(
            a.reshape(NCORES, nchunks, 128).transpose(0, 2, 1))

    cid_all = pack(cid_all)
    iota_np = np.broadcast_to(
        np.arange(G, dtype=ml_dtypes.bfloat16), (128, G)).copy()

    t1 = time.time()
    nc = _build_program(cpg)

    ndma_core = GROUPS_PER_CORE // GPD
    in_maps = []
    for k in range(NCORES):
        m = {
            "x": x_all[k * ndma_core:(k + 1) * ndma_core],
            "recip": recip_all[k],
        }
        if SHIP_OH:
            m["oh"] = oh_all[k * ndma_core:(k + 1) * ndma_core]
        else:
            m["cid"] = cid_all[k]
            m["iota"] = iota_np
        if QUANT == "int8":
            m["sg"] = sg_all[k]
        in_maps.append(m)
    print(f"[kernel] host prep {t1 - t0:.2f}s  build+compile "
          f"{time.time() - t1:.2f}s  (cpg={cpg}, quant={QUANT})",
          file=sys.stderr, flush=True)
    return nc, in_maps, cpg, unperm


def kernel(output: np.ndarray, mapping: np.ndarray) -> np.ndarray:
    nc, in_maps, _, unperm = _prepare(output, mapping)
    t2 = time.time()
    res = run_bass_kernel_spmd(nc, in_maps, list(range(NCORES)))
    t3 = time.time()
    full = np.concatenate([res.results[k]["out"] for k in range(NCORES)],
                          axis=0)                   # [4096, 256] device order
    full = full[unperm]                             # -> cluster order
    out = np.ascontiguousarray(full.T).reshape(32, 8, N_CLUSTERS)
    print(f"[kernel] run {t3 - t2:.2f}s", file=sys.stderr, flush=True)
    return out
